# revision 1
# baseline (speedup 1.0000x reference)
"""Trainium2 Bass kernel for nn_DistillSTU (LDS scan + spectral contraction).

Math: out[t,d] = sum_{delta>=0} k[delta,d] * u[t-delta,d],  u = x @ M_inputs,
      k[delta,d] = sum_j W[j,d]*Bm[j]*A[j]^delta (+ dvg[d] at delta=0),
      W = (C[:,:24]+C[:,24:]) @ M_filters, dvg = (Dv[:24]+Dv[24:]) @ M_filters.

Sharding: 768 channels split across 8 cores (96 each); embarrassingly parallel.

Per-core decomposition over T=2048 (chunks L=128, subs l=8):
  base   same-sub pairs (lag 0..7): exact short kernel; shift-FMA on
         ScalarE (lag 0) + GpSimd/VectorE (lags 1..7), d-partition layout.
  sub    same-chunk earlier-sub pairs: reduced-pole (r=8) states, batched
         across all chunks into 3 wide matmuls; one carry matmul per chunk.
  chunk  earlier-chunk pairs: exact 100 poles; chunk states via 3 wide
         matmuls + one tensor_tensor_scan; one carry matmul per chunk.
All state tensors use the (d,c)-interleaved free layout (col = d*NCH + c)
so per-chunk slices are stride-NCH column views.
"""
import sys
import numpy as np

sys.path.insert(0, "/opt/trn_rl_repo")

T = 2048
D = 768
NJ = 32           # reduced chunk-path state dim (fit is ~1e-11 exact)
L = 128           # chunk length
NCH = T // L      # 16 chunks
SUB = 8           # sub length
NS = L // SUB     # 16 subs per chunk
R = 8             # reduced poles for sub-carries; (s,p) = 15*8 = 120 <= 128
NCORE = 8
DP = D // NCORE   # 96 channels per core
FC = DP * NCH     # 1536 free cols of the (d,c) layout

_CACHE = {}

# column offsets inside the packed constant blocks (partition dim = 128)
_CONST_WIDTHS = [
    ("mi", 6 * DP), ("qt", NJ), ("pt4", 4 * L), ("rt", (NS - 1) * R),
    ("p2", (NS - 1) * SUB), ("ktab", SUB), ("ident", DP),
]
_CONST2_WIDTHS = [("gate", FC), ("wrep", FC), ("vrep", FC)]
CONST_OFF = {}
_off = 0
for _n, _w in _CONST_WIDTHS:
    CONST_OFF[_n] = _off
    _off += _w
CW = _off
CONST2_OFF = {}
_off = 0
for _n, _w in _CONST2_WIDTHS:
    CONST2_OFF[_n] = _off
    _off += _w
CW2 = _off


def _derive_tables(A, Bm, C, Dv, M_filters, M_inputs):
    """All host-side parameter preprocessing (small tensors only)."""
    f8 = np.float64
    A = A.astype(f8); Bm = Bm.astype(f8)
    C = C.astype(f8); Dv = Dv.astype(f8); Mf = M_filters.astype(f8)
    W = (C[:, :24] + C[:, 24:]) @ Mf                    # (100, 768)
    dvg = (Dv[:24] + Dv[24:]) @ Mf                      # (768,)
    V100 = W * Bm[:, None]                              # (100, 768)

    # exact short kernel (lags 0..7)
    pows = A[None, :] ** np.arange(SUB)[:, None]        # (8, 100)
    ktab8 = pows @ V100                                 # (8, 768)
    ktab8[0] += dvg

    # reduced-pole fit of k[delta,d] on delta in [1, L-1]; pole decay
    # rates refined by Nelder-Mead on the least-squares residual
    deltas = np.arange(1, L)
    kwin = (A[None, :] ** deltas[:, None]) @ V100       # (127, 768)

    def _fit(lam):
        mu = np.exp(-np.abs(lam))
        G = mu[None, :] ** deltas[:, None]
        Vr, *_ = np.linalg.lstsq(G, kwin, rcond=None)
        return mu, G, Vr, np.linalg.norm(G @ Vr - kwin)

    lam = np.geomspace(0.02, 1.5, R)
    mu, G, Vr, r0 = _fit(lam)
    try:
        from scipy.optimize import minimize
        res = minimize(lambda x: _fit(x)[3], lam, method="Nelder-Mead",
                       options={"maxiter": 3000, "fatol": 1e-12})
        mu2, G2, Vr2, r2 = _fit(res.x)
        if r2 < r0:
            mu, G, Vr = mu2, G2, Vr2
    except Exception:
        pass

    # chunk-level tables: 32 reduced poles fit on lags [1, 2047]
    d2 = np.arange(1, T)
    k2 = (A[None, :] ** d2[:, None]) @ V100             # (2047, 768)
    mu2 = np.exp(-np.geomspace(0.008, 3.0, NJ))
    G2 = mu2[None, :] ** d2[:, None]
    V2, *_ = np.linalg.lstsq(G2, k2, rcond=None)        # (32, 768)
    qt = mu2[None, :] ** (L - 1 - np.arange(L))[:, None]        # (128, 32)
    pt4 = np.zeros((4 * NJ, 4 * L))                     # block-diag carries
    ptb = np.ascontiguousarray((mu2[None, :] ** (np.arange(L) + 1)[:, None]).T)
    for c4 in range(4):
        pt4[c4 * NJ:(c4 + 1) * NJ, c4 * L:(c4 + 1) * L] = ptb
    gate = np.broadcast_to((mu2 ** L)[:, None], (NJ, FC)).copy()
    gate[:, 0::NCH] = 0.0                               # reset at c==0 per channel

    # sub-level tables (reduced poles); (s,p) order: s=1..15 outer, p inner
    rt = np.zeros((L, (NS - 1) * R))
    for s in range(1, NS):
        m = np.arange(SUB * s)
        rt[: SUB * s, (s - 1) * R:s * R] = mu[None, :] ** (SUB * s - 1 - m)[:, None]
    p2 = np.zeros(((NS - 1) * R, (NS - 1) * SUB))       # block-diag carries
    pr = mu[:, None] ** (np.arange(SUB) + 1)[None, :]   # (R, 8)
    for s in range(NS - 1):
        p2[s * R:(s + 1) * R, s * SUB:(s + 1) * SUB] = pr

    f4 = np.float32
    per_core = []
    for i in range(NCORE):
        sl = slice(i * DP, (i + 1) * DP)
        wrep = np.repeat(V2[:, sl][:, :, None], NCH, axis=2).reshape(NJ, FC)
        vrep = np.zeros(((NS - 1) * R, FC))
        vr_dc = np.repeat(Vr[:, sl][:, :, None], NCH, axis=2).reshape(R, FC)
        for s in range(NS - 1):
            vrep[s * R:(s + 1) * R] = vr_dc
        mi = np.ascontiguousarray(M_inputs.astype(f8)[:, sl]).astype(f4)
        ktabT = np.ascontiguousarray(ktab8[:, sl].T)    # (96, 8)
        cb = np.zeros((128, CW), dtype=f4)
        for name, arr in (
            ("mi", mi.reshape(6, 128, DP).transpose(1, 0, 2).reshape(128, 6 * DP)),
            ("qt", qt), ("pt4", pt4), ("rt", rt), ("p2", p2), ("ktab", ktabT),
            ("ident", np.eye(DP)),
        ):
            c0 = CONST_OFF[name]
            cb[:arr.shape[0], c0:c0 + arr.shape[1]] = arr
        cb2 = np.zeros((128, CW2), dtype=f4)
        for name, arr in (("gate", gate), ("wrep", wrep), ("vrep", vrep)):
            c0 = CONST2_OFF[name]
            cb2[:arr.shape[0], c0:c0 + arr.shape[1]] = arr
        per_core.append(dict(consts=cb, consts2=cb2))
    return per_core


def _build_nc():
    from concourse import bass, bacc, mybir, tile

    nc = bacc.Bacc()
    f4 = mybir.dt.float32
    xT = nc.declare_dram_parameter("xT", [D, T], f4, isOutput=False)
    cdram = nc.declare_dram_parameter("consts", [128, CW], f4, isOutput=False)
    cdram2 = nc.declare_dram_parameter("consts2", [128, CW2], f4, isOutput=False)
    out = nc.declare_dram_parameter("out", [DP, T], f4, isOutput=True)

    KT = D // L   # 6 k-tiles for the projection contraction
    NSPL = T // 512

    with tile.TileContext(nc) as tc:
        with (
            tc.tile_pool(name="consts", bufs=1) as consts,
            tc.tile_pool(name="xt", bufs=1) as xtp,
            tc.tile_pool(name="work", bufs=1) as work,
            tc.tile_pool(name="ps", bufs=2, space="PSUM") as psp,
            tc.tile_pool(name="big", bufs=3, space="PSUM") as bigp,
            tc.tile_pool(name="carry", bufs=3, space="PSUM") as cpool,
        ):
            call = consts.tile([128, CW], f4, tag="call")
            nc.sync.dma_start(call[:], cdram[:])
            call2 = consts.tile([128, CW2], f4, tag="call2")
            nc.sync.dma_start(call2[:], cdram2[:])

            def cs(name, rows, width, woff=0):
                c0 = CONST_OFF[name] + woff
                return call[0:rows, c0:c0 + width]

            def cs2(name, rows, width):
                c0 = CONST2_OFF[name]
                return call2[0:rows, c0:c0 + width]

            mi_sb = [cs("mi", 128, DP, k * DP) for k in range(KT)]
            qt_sb = cs("qt", L, NJ)
            pt4_sb = cs("pt4", 4 * NJ, 4 * L)
            rt_sb = cs("rt", L, (NS - 1) * R)
            p2_sb = cs("p2", (NS - 1) * R, (NS - 1) * SUB)
            ktab_sb = cs("ktab", DP, SUB)
            id_sb = cs("ident", DP, DP)
            gate_sb = cs2("gate", NJ, FC)
            wrep_sb = cs2("wrep", NJ, FC)
            vrep_sb = cs2("vrep", (NS - 1) * R, FC)

            xt_sb = []
            for k in range(KT):
                t = xtp.tile([L, T], f4, tag=f"xt{k}", name=f"xt_sb{k}")
                nc.sync.dma_start(t[:], xT[k * L:(k + 1) * L, :])
                xt_sb.append(t)

            u_dt = work.tile([DP, T], f4, tag="u_dt")
            u_tp = work.tile([L, FC], f4, tag="u_tp")
            s_all = work.tile([NJ, FC], f4, tag="s_all")
            e_all = work.tile([NJ, FC], f4, tag="e_all")
            f_all = work.tile([NJ, FC], f4, tag="f_all")
            f2_all = work.tile([(NS - 1) * R, FC], f4, tag="f2_all")
            f4sh = work.tile([4 * NJ, 4 * DP], f4, tag="f4sh")
            base_sb = work.tile([DP, T], f4, tag="base_sb")
            out_sb = work.tile([DP, T], f4, tag="out_sb")

            # ---- projection: u_dt[d, t] = sum_e mi[e, d] * xT[e, t]
            for n in range(NSPL):
                pu = psp.tile([DP, 512], f4, tag="ps")
                for k in range(KT):
                    nc.tensor.matmul(
                        pu[:], mi_sb[k], xt_sb[k][:, n * 512:(n + 1) * 512],
                        start=(k == 0), stop=(k == KT - 1))
                nc.scalar.copy(u_dt[:, n * 512:(n + 1) * 512], pu[:])

            # ---- base triangle (exact, lags 0..7) in d-partition layout.
            nc.scalar.activation(base_sb[:], u_dt[:],
                                 mybir.ActivationFunctionType.Copy,
                                 scale=ktab_sb[:, 0:1])
            for dlt in range(1, SUB):
                ov = base_sb[:].rearrange(
                    "d (sb l) -> d sb l", l=SUB)[:, :, dlt:SUB]
                uv = u_dt[:].rearrange(
                    "d (sb l) -> d sb l", l=SUB)[:, :, 0:SUB - dlt]
                nc.vector.scalar_tensor_tensor(
                    ov, uv, ktab_sb[:, dlt:dlt + 1], ov,
                    op0=mybir.AluOpType.mult, op1=mybir.AluOpType.add)

            # ---- u_tp: per-chunk transpose of u_dt, (d,c)-interleaved cols
            for c in range(NCH):
                ptp = psp.tile([L, DP], f4, tag="ps")
                nc.tensor.transpose(ptp[:], u_dt[:, c * L:(c + 1) * L], id_sb)
                nc.scalar.copy(u_tp[:, c::NCH], ptp[:])

            # ---- chunk states: 3 bank-sized matmuls + scan
            for n in range(3):
                sp = bigp.tile([NJ, 512], f4, tag="big", name=f"sp{n}")
                nc.tensor.matmul(sp[:], qt_sb,
                                 u_tp[:, n * 512:(n + 1) * 512],
                                 start=True, stop=True)
                nc.scalar.copy(s_all[:, n * 512:(n + 1) * 512], sp[:])
            nc.vector.tensor_tensor_scan(
                e_all[:], gate_sb, s_all[:], 0.0,
                op0=mybir.AluOpType.mult, op1=mybir.AluOpType.add)
            # f_all written (c,d)-blocked so chunk slices are contiguous
            nc.vector.tensor_tensor(
                f_all[:].rearrange("p (c d) -> p d c", d=DP),
                e_all[:].rearrange("p (d c) -> p d c", c=NCH),
                wrep_sb.rearrange("p (d c) -> p d c", c=NCH),
                op=mybir.AluOpType.mult)

            # ---- sub states: 3 bank-sized matmuls + fold fitted weights
            for n in range(3):
                ep = bigp.tile([(NS - 1) * R, 512], f4, tag="big", name=f"ep{n}")
                nc.tensor.matmul(ep[:], rt_sb,
                                 u_tp[:, n * 512:(n + 1) * 512],
                                 start=True, stop=True)
                nc.vector.tensor_tensor(
                    f2_all[:, n * 512:(n + 1) * 512], ep[:],
                    vrep_sb[:, n * 512:(n + 1) * 512],
                    op=mybir.AluOpType.mult)

            # ---- shuffle chunk states for 4x-batched carry matmuls:
            # f4sh[(c4, p), (g, d)] = f_all[p, (c=4g+c4-1, d)], zeros at c=0
            f4v = f4sh[:].rearrange("q (g d) -> q g d", d=DP)
            nc.vector.memset(f4sh[0:NJ, 0:DP], 0.0)
            fav = f_all[:].rearrange("p (c d) -> p c d", d=DP)
            nc.sync.dma_start(f4v[0:NJ, 1:4, :], fav[:, 3:12:4, :])
            for c4 in range(1, 4):
                nc.sync.dma_start(f4v[c4 * NJ:(c4 + 1) * NJ, :, :],
                                  fav[:, (c4 - 1)::4, :])

            # ---- per 4-chunk group: batched chunk carry + 4 sub carries,
            # then merge each chunk with the base and stream the output out
            for g in range(4):
                sacc = cpool.tile([DP, 4 * L], f4, tag="sacc", bufs=3)
                nc.tensor.matmul(sacc[:], f4sh[:, g * DP:(g + 1) * DP],
                                 pt4_sb, start=True, stop=False)
                for c4 in range(4):
                    c = 4 * g + c4
                    nc.tensor.matmul(
                        sacc[:, c4 * L + SUB:(c4 + 1) * L],
                        f2_all[:, c::NCH], p2_sb,
                        start=False, stop=(c4 == 3))
                for c4 in range(4):
                    c = 4 * g + c4
                    nc.vector.tensor_tensor(
                        out_sb[:, c * L:(c + 1) * L],
                        sacc[:, c4 * L:(c4 + 1) * L],
                        base_sb[:, c * L:(c + 1) * L], op=mybir.AluOpType.add)
                nc.sync.dma_start(out[:, g * 4 * L:(g + 1) * 4 * L],
                                  out_sb[:, g * 4 * L:(g + 1) * 4 * L])
    nc.compile()
    return nc


def _get_program():
    if "nc" not in _CACHE:
        _CACHE["nc"] = _build_nc()
    return _CACHE["nc"]


def kernel(x, input_pos, M_inputs, M_filters, A, Bm, C, Dv, _trace=False,
           _trace_kwargs=None):
    from concourse.bass_utils import run_bass_kernel_spmd

    x = np.asarray(x, dtype=np.float32)
    per_core = _derive_tables(
        np.asarray(A), np.asarray(Bm), np.asarray(C), np.asarray(Dv),
        np.asarray(M_filters), np.asarray(M_inputs))
    xTm = np.ascontiguousarray(x[0].T)                   # (768, 2048)

    nc = _get_program()
    in_maps = [dict(xT=xTm, **per_core[i]) for i in range(NCORE)]
    kw = dict(_trace_kwargs or {})
    res = run_bass_kernel_spmd(nc, in_maps, list(range(NCORE)),
                               trace=_trace, **kw)
    _CACHE["last_result"] = res
    full = np.concatenate([res.results[i]["out"] for i in range(NCORE)], axis=0)
    return np.ascontiguousarray(full.T)[None].astype(np.float32)


if __name__ == "__main__":
    rng = np.random.default_rng(0)
    ins = dict(
        x=rng.standard_normal((1, T, D), dtype=np.float32),
        input_pos=np.arange(T, dtype=np.int32),
        M_inputs=(rng.standard_normal((D, D)) * 0.02).astype(np.float32),
        M_filters=(rng.standard_normal((24, D)) * 0.02).astype(np.float32),
        A=rng.uniform(0, 0.99, 100).astype(np.float32),
        Bm=(rng.standard_normal(100) * 0.1).astype(np.float32),
        C=(rng.standard_normal((100, 48)) * 0.1).astype(np.float32),
        Dv=(rng.standard_normal(48) * 0.1).astype(np.float32),
    )
    got = kernel(**ins)
    print("kernel output", got.shape, got.dtype, float(np.abs(got).max()))



# revision 9
# speedup vs baseline: 1.7747x; 1.7747x over previous
"""Trainium2 Bass kernel for nn_DistillSTU (LDS scan + spectral contraction).

Math: out[t,d] = sum_{delta>=0} k[delta,d] * u[t-delta,d],  u = x @ M_inputs,
      k[delta,d] = sum_j W[j,d]*Bm[j]*A[j]^delta (+ dvg[d] at delta=0),
      W = (C[:,:24]+C[:,24:]) @ M_filters, dvg = (Dv[:24]+Dv[24:]) @ M_filters.

Sharding: 768 channels split across 8 cores (96 each); embarrassingly parallel.

Per-core decomposition over T=2048 (chunks L=128, subs l=8), all-bf16 data:
  base   same-sub pairs (lag 0..7): exact short kernel; shift-FMA split
         across Scalar (lag 0) / Vector (1-4) / GpSimd (5-7), d-partition;
         own DRAM output, summed with the carries on host.
  sub    same-chunk earlier-sub pairs: reduced-pole (R=8) states from the
         combined [qt|rt] state matmul; carries via stationary p2f with
         f2 streamed -> t-partition PSUM.
  chunk  earlier-chunk pairs: NJ=8 refined poles; gated scan over chunk
         boundaries (split V/G), then stationary ptb streams the
         shifted+weighted states into the same t-partition PSUM.
Carry output is [t_in_chunk, (c,d)]; host reassembles + adds base.
u transposes (d-part -> t-part per chunk) run on the DMA XBAR.
"""
import sys
import numpy as np

sys.path.insert(0, "/opt/trn_rl_repo")

T = 2048
D = 768
NJ = 8            # chunk-path reduced poles (refined fit, ~5e-4)
L = 128           # chunk length
NCH = T // L      # 16 chunks
SUB = 8           # sub length
NS = L // SUB     # 16 subs per chunk
R = 8             # sub-path reduced poles; (s,p) = 15*8 = 120
NCORE = 8
DP = D // NCORE   # 96 channels per core
FC = DP * NCH     # 1536 = (c,d)/(d,c) interleaved free width
XW = 6 * 512      # per-time-block xt width (6 k-tiles x 512 cols)

_CACHE = {}

# column offsets inside the packed bf16 constant block (partition dim 128)
_CONST_WIDTHS = [
    ("mi", 6 * DP),        # projection weights, k-major tiles
    ("qt", NJ),            # chunk-state weights (qt)
    ("rt", 120),           # sub-state weights (rt)
    ("p2f", L),            # sub-carry propagation (stationary), 8 zero cols
    ("ptb", L),            # chunk-carry propagation (stationary, rows 0:8)
    ("w2", DP),            # chunk-pole output weights seed (rows 0:8)
    ("vr", DP),            # sub-pole output weights seed (rows 0:120)
    ("gs", 2 * NCH),       # fp32 scan gate seed, bit-packed (rows 0:8)
    ("ktab", 2 * SUB),     # fp32 ktab, bit-packed as bf16 pairs (rows 0:96)
]
CONST_OFF = {}
_off = 0
for _n, _w in _CONST_WIDTHS:
    CONST_OFF[_n] = _off
    _off += _w
CW = _off


def _derive_tables(A, Bm, C, Dv, M_filters, M_inputs):
    """All host-side parameter preprocessing (small tensors only)."""
    import ml_dtypes
    bf16 = ml_dtypes.bfloat16
    f8 = np.float64
    A = A.astype(f8); Bm = Bm.astype(f8)
    C = C.astype(f8); Dv = Dv.astype(f8); Mf = M_filters.astype(f8)
    W = (C[:, :24] + C[:, 24:]) @ Mf                    # (100, 768)
    dvg = (Dv[:24] + Dv[24:]) @ Mf                      # (768,)
    V100 = W * Bm[:, None]                              # (100, 768)

    # exact short kernel (lags 0..7)
    pows = A[None, :] ** np.arange(SUB)[:, None]        # (8, 100)
    ktab8 = pows @ V100                                 # (8, 768)
    ktab8[0] += dvg

    def _fit(lam, dl, kt):
        mu = np.exp(-np.abs(lam))
        G = mu[None, :] ** dl[:, None]
        Vr, *_ = np.linalg.lstsq(G, kt, rcond=None)
        return mu, G, Vr, np.linalg.norm(G @ Vr - kt)

    def _refine(lam0, dl, kt, iters):
        mu, G, Vr, r0 = _fit(lam0, dl, kt)
        try:
            from scipy.optimize import minimize
            res = minimize(lambda x: _fit(x, dl, kt)[3], lam0,
                           method="Nelder-Mead",
                           options={"maxiter": iters, "fatol": 1e-13})
            mu2, G2, Vr2, r2 = _fit(res.x, dl, kt)
            if r2 < r0:
                return mu2, Vr2
        except Exception:
            pass
        return mu, Vr

    # sub path: reduced-pole fit of k[delta,d] on delta in [1, L-1]
    deltas = np.arange(1, L)
    kwin = (A[None, :] ** deltas[:, None]) @ V100       # (127, 768)
    mu, Vr = _refine(np.geomspace(0.02, 1.5, R), deltas, kwin, 1500)

    # chunk path: NJ refined poles on lags [1, 2047]
    d2 = np.arange(1, T)
    k2 = (A[None, :] ** d2[:, None]) @ V100             # (2047, 768)
    mu2, V2 = _refine(np.geomspace(0.008, 3.0, NJ), d2, k2, 800)

    # state weights, chunk-relative time rows
    qt = mu2[None, :] ** (L - 1 - np.arange(L))[:, None]        # (128, 8)
    rt = np.zeros((L, (NS - 1) * R))
    for s in range(1, NS):
        m = np.arange(SUB * s)
        rt[: SUB * s, (s - 1) * R: s * R] = \
            mu[None, :] ** (SUB * s - 1 - m)[:, None]

    # sub-carry propagation, stationary: rows (s,p) -> cols j' of next sub
    p2f = np.zeros(((NS - 1) * R, L))
    pr = mu[:, None] ** (np.arange(SUB) + 1)[None, :]   # (R, 8)
    for s in range(NS - 1):
        p2f[s * R:(s + 1) * R, (s + 1) * SUB:(s + 2) * SUB] = pr

    # chunk-carry propagation, stationary: ptb[p, j'] = mu2_p^(j'+1)
    ptb = np.ascontiguousarray(mu2[:, None] ** (np.arange(L) + 1)[None, :])

    # scan gate seed [NJ, NCH] fp32: 0 at c==0 (channel reset), mu2^L else
    gs = np.broadcast_to((mu2 ** L)[:, None], (NJ, NCH)).copy()
    gs[:, 0] = 0.0

    f4 = np.float32
    per_core = []
    for i in range(NCORE):
        sl = slice(i * DP, (i + 1) * DP)
        mi = np.ascontiguousarray(M_inputs.astype(f8)[:, sl])
        cb = np.zeros((128, CW), dtype=bf16)

        def put(name, arr):
            c0 = CONST_OFF[name]
            a = np.asarray(arr)
            cb[:a.shape[0], c0:c0 + a.shape[1]] = a.astype(bf16)

        put("mi", mi.reshape(6, 128, DP).transpose(1, 0, 2).reshape(128, 6 * DP))
        put("qt", qt)
        put("rt", rt)
        put("p2f", p2f)
        put("ptb", ptb)
        put("w2", V2[:, sl])                            # (8, 96)
        put("vr", np.tile(Vr[:, sl], (NS - 1, 1)))      # (120, 96)
        # fp32 tables bit-packed into bf16 columns (2 bf16 per fp32)
        c0 = CONST_OFF["gs"]
        cb[:NJ, c0:c0 + 2 * NCH] = gs.astype(f4).view(bf16)
        ktabT = np.ascontiguousarray(ktab8[:, sl].T).astype(f4)   # (96, 8)
        c0 = CONST_OFF["ktab"]
        cb[:DP, c0:c0 + 2 * SUB] = ktabT.view(bf16)
        per_core.append(dict(cb=cb))
    return per_core


def _build_nc(transp="xbar"):
    from concourse import bass, bacc, mybir, tile

    nc = bacc.Bacc()
    f4 = mybir.dt.float32
    bf = mybir.dt.bfloat16
    xb = nc.declare_dram_parameter("xb", [128, 4 * XW], bf, isOutput=False)
    cdram = nc.declare_dram_parameter("cb", [128, CW], bf, isOutput=False)
    outc = nc.declare_dram_parameter("outc", [L, FC], bf, isOutput=True)
    outb = nc.declare_dram_parameter("outb", [DP, T], bf, isOutput=True)

    KT = 6
    Alu = mybir.AluOpType
    Act = mybir.ActivationFunctionType

    with tile.TileContext(nc) as tc:
        with (
            tc.tile_pool(name="consts", bufs=1) as consts,
            tc.tile_pool(name="xt", bufs=1) as xtp,
            tc.tile_pool(name="work", bufs=1) as work,
            tc.tile_pool(name="ups", bufs=2, space="PSUM") as upsp,
            tc.tile_pool(name="stq", bufs=1, space="PSUM") as stqp,
            tc.tile_pool(name="str", bufs=2, space="PSUM") as strp,
            tc.tile_pool(name="sacc", bufs=3, space="PSUM") as saccp,
        ):
            call = consts.tile([128, CW], bf, tag="call")
            nc.scalar.dma_start(call[:], cdram[:])

            def cs(name, rows, width, woff=0):
                c0 = CONST_OFF[name] + woff
                return call[0:rows, c0:c0 + width]

            mi_sb = [cs("mi", 128, DP, k * DP) for k in range(KT)]
            qt_sb = cs("qt", 128, NJ)
            rt_sb = cs("rt", 128, 120)
            p2f_sb = cs("p2f", (NS - 1) * R, L)
            ptb_sb = cs("ptb", NJ, L)
            w2_sb = cs("w2", NJ, DP)
            vr_sb = cs("vr", (NS - 1) * R, DP)
            gs_sb = cs("gs", NJ, 2 * NCH).bitcast(f4)       # (8, 16) fp32
            ktab_sb = cs("ktab", DP, 2 * SUB).bitcast(f4)   # (96, 8) fp32

            xt_sb = xtp.tile([128, 4 * XW], bf, tag="xt")
            for n in range(4):
                nc.sync.dma_start(xt_sb[:, n * XW:(n + 1) * XW],
                                  xb[:, n * XW:(n + 1) * XW])

            u_dt = work.tile([DP, T], bf, tag="u_dt")
            u_tp = work.tile([L, FC], bf, tag="u_tp")
            base_sb = work.tile([DP, T], bf, tag="base_sb")
            s_all = work.tile([NJ, FC], f4, tag="s_all")
            e_all = work.tile([NJ, FC], bf, tag="e_all")
            fsh = work.tile([NJ, FC], bf, tag="fsh")
            f2_all = work.tile([(NS - 1) * R, FC], bf, tag="f2_all")
            outc_sb = work.tile([L, FC], bf, tag="outc_sb")
            gate_sb = work.tile([NJ, FC], f4, tag="gate_sb")
            saccs = [saccp.tile([L, 512], f4, tag="sacc", name=f"sacc{b}")
                     for b in range(3)]

            # materialize the scan gate by broadcasting the fp32 seed
            nc.vector.tensor_copy(
                gate_sb[:].rearrange("p (d c) -> p d c", c=NCH),
                gs_sb.unsqueeze(1).broadcast_to([NJ, DP, NCH]))

            def base_block(n):
                blk = slice(n * 512, (n + 1) * 512)
                nc.scalar.activation(base_sb[:, blk], u_dt[:, blk],
                                     Act.Copy, scale=ktab_sb[:, 0:1])
                for dlt in range(1, SUB):
                    eng = nc.vector
                    ov = base_sb[:, blk].rearrange(
                        "d (sb l) -> d sb l", l=SUB)[:, :, dlt:SUB]
                    uv = u_dt[:, blk].rearrange(
                        "d (sb l) -> d sb l", l=SUB)[:, :, 0:SUB - dlt]
                    eng.scalar_tensor_tensor(
                        ov, uv, ktab_sb[:, dlt:dlt + 1], ov,
                        op0=Alu.mult, op1=Alu.add)
                nc.sync.dma_start(outb[:, blk], base_sb[:, blk])

            GW = 4 * DP     # 384-col chunk group

            def states_group(g):
                gb = slice(g * GW, (g + 1) * GW)
                sq = stqp.tile([NJ, GW], f4, tag="stq")
                sr = strp.tile([120, GW], f4, tag="str")
                nc.tensor.matmul(sq[:], qt_sb, u_tp[:, gb],
                                 start=True, stop=True)
                nc.tensor.matmul(sr[:], rt_sb, u_tp[:, gb],
                                 start=True, stop=True)
                # f2 = ep * vr (broadcast over c)
                nc.vector.tensor_tensor(
                    f2_all[:, gb].rearrange("p (c d) -> p c d", d=DP),
                    sr[:].rearrange("p (c d) -> p c d", d=DP),
                    vr_sb.unsqueeze(1).broadcast_to([(NS - 1) * R, 4, DP]),
                    op=Alu.mult)
                # chunk-state copy into scan layout (d,c)
                nc.vector.tensor_copy(
                    s_all[:].rearrange("p (d c) -> p c d", c=NCH)[
                        :, 4 * g:4 * g + 4, :],
                    sq[:].rearrange("p (c d) -> p c d", d=DP))

            # ---- pipeline ----
            for n in range(4):
                pu = upsp.tile([DP, 512], f4, tag="pu")
                for k in range(KT):
                    nc.tensor.matmul(
                        pu[:], mi_sb[k],
                        xt_sb[:, n * XW + k * 512: n * XW + (k + 1) * 512],
                        start=(k == 0), stop=(k == KT - 1))
                nc.scalar.copy(u_dt[:, n * 512:(n + 1) * 512], pu[:])
                for c in range(4 * n, 4 * n + 4):
                    src = u_dt[:, c * L:(c + 1) * L]
                    dst = u_tp[:, c * DP:(c + 1) * DP]
                    eng = nc.scalar if c % 2 == 0 else nc.sync
                    eng.dma_start(dst, src, transpose=True)
                if n >= 1:
                    states_group(n - 1)   # group n-1 fully transposed
                if n < 3:
                    base_block(n)

            states_group(3)
            # sub carries into t-partition PSUM (stationary p2f)
            for b in range(3):
                nc.tensor.matmul(saccs[b][:], p2f_sb,
                                 f2_all[:, b * 512:(b + 1) * 512],
                                 start=True, stop=False)
            # gated scan over chunks within each d
            nc.vector.tensor_tensor_scan(
                e_all[:], gate_sb[:], s_all[:],
                0.0, op0=Alu.mult, op1=Alu.add)
            # fsh[(c,d)] = e[(d,c-1)] * w2  (shift by one chunk)
            nc.vector.memset(fsh[:, 0:DP], 0.0)
            nc.vector.tensor_tensor(
                fsh[:, DP:FC].rearrange("p (c d) -> p c d", d=DP),
                e_all[:].rearrange("p (d c) -> p c d", c=NCH)[:, 0:NCH - 1, :],
                w2_sb.unsqueeze(1).broadcast_to([NJ, NCH - 1, DP]),
                op=Alu.mult)
            # chunk carries (stationary ptb), then copy + stream out
            for b in range(3):
                nc.tensor.matmul(saccs[b][:], ptb_sb,
                                 fsh[:, b * 512:(b + 1) * 512],
                                 start=False, stop=True)
                nc.scalar.copy(outc_sb[:, b * 512:(b + 1) * 512],
                               saccs[b][:])
                nc.scalar.dma_start(outc[:, b * 512:(b + 1) * 512],
                                    outc_sb[:, b * 512:(b + 1) * 512])
            base_block(3)
    nc.compile()
    return nc


def _get_program():
    if "nc" not in _CACHE:
        _CACHE["nc"] = _build_nc()
    return _CACHE["nc"]


def kernel(x, input_pos, M_inputs, M_filters, A, Bm, C, Dv, _trace=False,
           _trace_kwargs=None):
    import ml_dtypes
    from concourse.bass_utils import run_bass_kernel_spmd

    bf16 = ml_dtypes.bfloat16
    x = np.asarray(x, dtype=np.float32)
    per_core = _derive_tables(
        np.asarray(A), np.asarray(Bm), np.asarray(C), np.asarray(Dv),
        np.asarray(M_filters), np.asarray(M_inputs))
    # xb[p, n*XW + k*512 + j] = x[n*512 + j, k*128 + p]  (bf16)
    xT = np.ascontiguousarray(x[0].T.astype(bf16))       # (768, 2048)
    xbm = np.ascontiguousarray(
        xT.reshape(6, 128, 4, 512).transpose(1, 2, 0, 3).reshape(128, 4 * XW))

    nc = _get_program()
    in_maps = [dict(xb=xbm, **per_core[i]) for i in range(NCORE)]
    kw = dict(_trace_kwargs or {})
    res = run_bass_kernel_spmd(nc, in_maps, list(range(NCORE)),
                               trace=_trace, **kw)
    _CACHE["last_result"] = res
    full = np.empty((T, D), dtype=np.float32)
    for i in range(NCORE):
        oc = np.asarray(res.results[i]["outc"]).astype(np.float32)
        ob = np.asarray(res.results[i]["outb"]).astype(np.float32)
        # outc: [j', (c,d)] -> (T, DP); outb: [d, t] -> (T, DP)
        carr = oc.reshape(L, NCH, DP).transpose(1, 0, 2).reshape(T, DP)
        full[:, i * DP:(i + 1) * DP] = carr + ob.T
    return full[None]


if __name__ == "__main__":
    rng = np.random.default_rng(0)
    ins = dict(
        x=rng.standard_normal((1, T, D), dtype=np.float32),
        input_pos=np.arange(T, dtype=np.int32),
        M_inputs=(rng.standard_normal((D, D)) * 0.02).astype(np.float32),
        M_filters=(rng.standard_normal((24, D)) * 0.02).astype(np.float32),
        A=rng.uniform(0, 0.99, 100).astype(np.float32),
        Bm=(rng.standard_normal(100) * 0.1).astype(np.float32),
        C=(rng.standard_normal((100, 48)) * 0.1).astype(np.float32),
        Dv=(rng.standard_normal(48) * 0.1).astype(np.float32),
    )
    got = kernel(**ins)
    print("kernel output", got.shape, got.dtype, float(np.abs(got).max()))
